# revision 41
# baseline (speedup 1.0000x reference)
"""CrossAttention kernel for 8 Trainium2 NeuronCores.

Data-parallel over batch: B=16 batches -> 2 per core. Each core computes the
full cross-attention for its 2 batches; outputs are concatenated on host.

K = ctx@Wk and V = ctx@Wv are tiny (0.15% of FLOPs) and computed on host
during input marshaling, uploaded pre-transposed in the layouts the PE wants.

Per-core dataflow (all matmuls f16 in, fp32 psum, full-rate on the PE):
  x      [2,4096,512] --host transpose--> x^T tiles [512, nq] (f16)
  Q^T    = Wq^T @ x^T            (Wq pre-scaled by 1/8 on host)
  S^T    = (K_h^T).T @ Q_h^T     [77, nq]  per head
  expS   = exp(S^T)              (no max-subtract; |S| <~ 6 so fp32 exp safe)
  den    = ones^T @ expS         [64, nq] per head, pairs packed [128, nq]
  O^T    = V_h.T @ expS          [64, nq], head pairs packed into [128, nq]
  ot     = O^T * (1/den)         (DVE recip + mul, per pair)
  out    = (O^T).T @ Wo + bo     [nq, 512], stored f16
"""

import os
import sys

for _p in ("/opt/trn_rl_repo",):
    if _p not in sys.path:
        sys.path.insert(0, _p)

import numpy as np

import concourse.bass as bass
import concourse.bass_isa as bass_isa
import concourse.bacc as bacc
import concourse.mybir as mybir
import concourse.tile as tile
from concourse.bass_utils import run_bass_kernel_spmd

# Problem constants (hardcoded per contract)
B, NQ, NK = 16, 4096, 77
DQ, DC = 512, 768
H, DH = 8, 64
INNER = H * DH  # 512
SCALE = DH ** -0.5  # 1/8
NCORES = 8
BLOC = B // NCORES  # 2 batches per core

F32 = mybir.dt.float32
F16 = mybir.dt.float16

TQ = 512          # nq tile (free dim of most matmuls)
NTILES = NQ // TQ  # 8 per batch

KQ = DQ // 128   # 4 contraction chunks for Wq
CI = INNER // 128  # 4 inner chunks


def _build_nc():
    nc = bacc.Bacc("TRN2", target_bir_lowering=False, debug=False)

    xT_l = nc.dram_tensor("xT_l", [BLOC, DQ, NQ], F16, kind="ExternalInput")
    kt_l = nc.dram_tensor("kt_l", [128, CI, BLOC * NK], F16,
                          kind="ExternalInput")
    v_l = nc.dram_tensor("v_l", [NK, BLOC, INNER], F16, kind="ExternalInput")
    # wq/wo uploaded pre-rearranged to [p, k, n] so the DMA is contiguous
    wq = nc.dram_tensor("wq", [128, KQ, INNER], F16, kind="ExternalInput")
    wo = nc.dram_tensor("wo", [128, CI, DQ], F16, kind="ExternalInput")
    ones77 = nc.dram_tensor("ones77", [NK, 64], F16, kind="ExternalInput")
    out_l = nc.dram_tensor("out_l", [BLOC, NQ, DQ], F16, kind="ExternalOutput")

    with tile.TileContext(nc) as tc:
        with (
            tc.tile_pool(name="consts", bufs=1) as consts,
            tc.tile_pool(name="xp", bufs=3) as xp,
            tc.tile_pool(name="ep", bufs=12) as ep,
            tc.tile_pool(name="rp", bufs=4) as rp,
            tc.tile_pool(name="op", bufs=2) as op,
            tc.tile_pool(name="fp", bufs=2) as fp,
            tc.tile_pool(name="ps", bufs=8, space="PSUM") as ps,
        ):
            # ---- DMAs: tile-0 Q-proj critical path first. Tile 0 is split
            # in half so the first matmul waits on a 256KB transfer only.
            TQ0 = TQ // 2
            xT0_sb = xp.tile([128, KQ, TQ0], F16, tag="xT")
            nc.sync.dma_start(
                xT0_sb[:],
                xT_l[0].rearrange("(c p) n -> p c n", p=128)[:, :, 0:TQ0])
            wq_sb = consts.tile([128, KQ, INNER], F16)
            nc.scalar.dma_start(wq_sb[:], wq[:])
            kt_sb = consts.tile([128, CI, BLOC * NK], F16)
            nc.sync.dma_start(kt_sb[:], kt_l[:])
            v_sb = consts.tile([NK, BLOC, INNER], F16)
            nc.sync.dma_start(v_sb[:], v_l[:])
            ones77_sb = consts.tile([NK, 64], F16)
            nc.gpsimd.dma_start(ones77_sb[:], ones77[:])
            wo_sb = consts.tile([128, CI, DQ], F16)
            nc.scalar.dma_start(wo_sb[:], wo[:])

            # ---- main loop over (batch, nq tile) ----
            def process_tile(idx, b, nq0, tq, xT_sb):
                nj = tq // 128
                # Q^T = Wq^T @ x^T (Wq pre-scaled by 1/8 on host), with the
                # S^T matmuls for head pair g interleaved right after Q chunk
                # g+1 so the scalar exp chain starts ~2us earlier and the
                # den/O matmuls never wait on it. qt psum->sbuf copies run on
                # the vector engine (the scalar engine is exp-saturated in
                # this phase).
                qt_sb = xp.tile([128, CI, tq], F16, tag="qt",
                                name=f"qt_{idx}")
                e_sbs = []

                def emit_s_pair(g):
                    for h in (2 * g, 2 * g + 1):
                        c, r = h // 2, (h % 2) * 64
                        s_ps = ps.tile([NK, tq], F32, tag="ps",
                                       name=f"sps_{idx}_{h}")
                        nc.tensor.matmul(
                            s_ps[:],
                            kt_sb[r:r + DH, c, b * NK:(b + 1) * NK],
                            qt_sb[r:r + DH, c, :])
                        e_sb = ep.tile([NK, tq], F16, tag="expS",
                                       name=f"e_{idx}_{h}")
                        nc.scalar.activation(
                            e_sb[:], s_ps[:],
                            mybir.ActivationFunctionType.Exp)
                        e_sbs.append(e_sb)

                ot_sb = op.tile([128, CI, tq], F16, tag="ot",
                                name=f"ot_{idx}")

                def emit_pair(g):
                    # dens first so the vector engine's recip->mul chain for
                    # this pair starts two matmuls earlier.
                    o2_ps = ps.tile([128, tq], F32, tag="ps",
                                    name=f"ops_{idx}_{g}")
                    d_ps = ps.tile([128, tq], F32, tag="ps",
                                   name=f"dps_{idx}_{g}")
                    for half in range(2):
                        nc.tensor.matmul(
                            d_ps[half * 64:(half + 1) * 64, :],
                            ones77_sb[:], e_sbs[2 * g + half][:],
                            tile_position=(0, half * 64))
                    for half in range(2):
                        h = 2 * g + half
                        nc.tensor.matmul(
                            o2_ps[half * 64:(half + 1) * 64, :],
                            v_sb[:, b, h * DH:(h + 1) * DH],
                            e_sbs[h][:])
                    rec_sb = rp.tile([128, tq], F32, tag="rec",
                                     name=f"rec_{idx}_{g}")
                    nc.vector.reciprocal_approx_fast(rec_sb[:], d_ps[:])
                    nc.vector.tensor_mul(
                        ot_sb[:, g, :], o2_ps[:], rec_sb[:])

                for c in range(CI):
                    q_ps = ps.tile([128, tq], F32, tag="ps",
                                   name=f"qps_{idx}_{c}")
                    for k in range(KQ):
                        nc.tensor.matmul(
                            q_ps[:], wq_sb[:, k, c * 128:(c + 1) * 128],
                            xT_sb[:, k, :],
                            start=(k == 0), stop=(k == KQ - 1))
                    nc.vector.tensor_copy(qt_sb[:, c, :], q_ps[:])
                    if c >= 1:
                        emit_s_pair(c - 1)
                # heads 6,7 wait on the qt c3 copy; slot pair 0's den/O
                # matmuls in between so the PE never idles on that copy.
                emit_pair(0)
                emit_s_pair(3)
                for g in range(1, H // 2):
                    emit_pair(g)

                # out = (O^T).T @ Wo + bo; c-major accumulation so each
                # pair's contribution starts as soon as its mul lands.
                f_pss = [ps.tile([128, DQ], F32, tag="ps",
                                 name=f"fps_{idx}_{j}")
                         for j in range(nj)]
                for c in range(CI):
                    for j in range(nj):
                        nc.tensor.matmul(
                            f_pss[j][:],
                            ot_sb[:, c, j * 128:(j + 1) * 128],
                            wo_sb[:, c, :],
                            start=(c == 0), stop=(c == CI - 1))
                # f32->f16 cast on the scalar engine (idle in this phase);
                # the +bo bias is applied on host after gathering. On the
                # final tile, split casts across scalar and vector to halve
                # the pipeline drain.
                last = idx == len(tiles) - 1
                f_sb = fp.tile([128, nj, DQ], F16, tag="fin",
                               name=f"fin_{idx}")
                for j in range(nj):
                    if last and j % 2 == 1:
                        nc.vector.tensor_copy(f_sb[:, j, :], f_pss[j][:])
                    else:
                        nc.scalar.activation(
                            f_sb[:, j, :], f_pss[j][:],
                            mybir.ActivationFunctionType.Copy)
                    st_eng = [nc.sync, nc.scalar, nc.gpsimd][
                        (nj * idx + j) % 3]
                    st_eng.dma_start(
                        out_l[b, nq0 + j * 128:nq0 + (j + 1) * 128, :]
                        .rearrange("p n -> p n"),
                        f_sb[:, j, :])

            tiles = [(0, 0, TQ0), (0, TQ0, TQ0)] + [
                (b, t * TQ, TQ)
                for b in range(BLOC) for t in range(NTILES)][1:]

            for idx, (b, nq0, tq) in enumerate(tiles):
                if idx == 0:
                    xT_sb = xT0_sb
                else:
                    xT_sb = xp.tile([128, KQ, tq], F16, tag="xT",
                                    name=f"xT_{idx}")
                    xT_src = xT_l[b].rearrange("(c p) n -> p c n", p=128)[
                        :, :, nq0:nq0 + tq]
                    if idx % 2 == 0:
                        nc.scalar.dma_start(xT_sb[:], xT_src)
                    else:
                        nc.sync.dma_start(xT_sb[:], xT_src)
                process_tile(idx, b, nq0, tq, xT_sb)

    nc.compile()
    return nc


_NC_CACHE = {}


def _get_nc():
    if "nc" not in _NC_CACHE:
        _NC_CACHE["nc"] = _build_nc()
    return _NC_CACHE["nc"]


def _make_in_maps(x, context, Wq, Wk, Wv, Wo, bo):
    f = np.float32
    ctx32 = np.ascontiguousarray(context, dtype=f)
    K_all = ctx32 @ np.asarray(Wk, dtype=f)   # [B, NK, INNER]
    V_all = ctx32 @ np.asarray(Wv, dtype=f)   # [B, NK, INNER]
    def fold_w(w, kdim, n):
        # [kdim*128, n] -> [128, kdim, n] contiguous
        return np.ascontiguousarray(
            w.reshape(kdim, 128, n).transpose(1, 0, 2))
    shared = {
        "wq": fold_w((np.asarray(Wq, dtype=f) * np.float32(SCALE)
                      ).astype(np.float16), KQ, INNER),
        "wo": fold_w(np.asarray(Wo, dtype=f).astype(np.float16), CI, DQ),
        "ones77": np.ones((NK, 64), dtype=np.float16),
    }
    in_maps = []
    for i in range(NCORES):
        m = dict(shared)
        m["xT_l"] = np.ascontiguousarray(
            np.asarray(x[BLOC * i:BLOC * (i + 1)], dtype=f).transpose(
                0, 2, 1)).astype(np.float16)
        K_c = K_all[BLOC * i:BLOC * (i + 1)]  # [BLOC, NK, INNER]
        # kt[p, c, b*NK+k] = K[b, k, c*128+p]
        m["kt_l"] = np.ascontiguousarray(
            K_c.transpose(2, 0, 1).reshape(CI, 128, BLOC, NK).transpose(
                1, 0, 2, 3).reshape(128, CI, BLOC * NK)).astype(np.float16)
        V_c = V_all[BLOC * i:BLOC * (i + 1)]
        m["v_l"] = np.ascontiguousarray(
            V_c.transpose(1, 0, 2)).astype(np.float16)
        in_maps.append(m)
    return in_maps


def run(x, context, Wq, Wk, Wv, Wo, bo, trace=False, **trace_kwargs):
    nc = _get_nc()
    in_maps = _make_in_maps(x, context, Wq, Wk, Wv, Wo, bo)
    res = run_bass_kernel_spmd(
        nc, in_maps, list(range(NCORES)), trace=trace, **trace_kwargs)
    out = np.concatenate(
        [np.asarray(res.results[i]["out_l"]) for i in range(NCORES)], axis=0)
    return out.astype(np.float32) + np.asarray(bo, dtype=np.float32), res


def kernel(x, context, Wq, Wk, Wv, Wo, bo):
    out, _ = run(x, context, Wq, Wk, Wv, Wo, bo, trace=False)
    return out


# revision 43
# speedup vs baseline: 1.0319x; 1.0319x over previous
"""CrossAttention kernel for 8 Trainium2 NeuronCores.

Data-parallel over batch: B=16 batches -> 2 per core. Each core computes the
full cross-attention for its 2 batches; outputs are concatenated on host.

K = ctx@Wk and V = ctx@Wv are tiny (0.15% of FLOPs) and computed on host
during input marshaling, uploaded pre-transposed in the layouts the PE wants.

Per-core dataflow (all matmuls f16 in, fp32 psum, full-rate on the PE):
  x      [2,4096,512] --host transpose--> x^T tiles [512, nq] (f16)
  Q^T    = Wq^T @ x^T            (Wq pre-scaled by 1/8 on host)
  S^T    = (K_h^T).T @ Q_h^T     [77, nq]  per head
  expS   = exp(S^T)              (no max-subtract; |S| <~ 6 so fp32 exp safe)
  den    = ones^T @ expS         [64, nq] per head, pairs packed [128, nq]
  O^T    = V_h.T @ expS          [64, nq], head pairs packed into [128, nq]
  ot     = O^T * (1/den)         (DVE recip + mul, per pair)
  out    = (O^T).T @ Wo + bo     [nq, 512], stored f16
"""

import os
import sys

for _p in ("/opt/trn_rl_repo",):
    if _p not in sys.path:
        sys.path.insert(0, _p)

import numpy as np

import concourse.bass as bass
import concourse.bass_isa as bass_isa
import concourse.bacc as bacc
import concourse.mybir as mybir
import concourse.tile as tile
from concourse.bass_utils import run_bass_kernel_spmd

# Problem constants (hardcoded per contract)
B, NQ, NK = 16, 4096, 77
DQ, DC = 512, 768
H, DH = 8, 64
INNER = H * DH  # 512
SCALE = DH ** -0.5  # 1/8
NCORES = 8
BLOC = B // NCORES  # 2 batches per core

F32 = mybir.dt.float32
F16 = mybir.dt.float16

TQ = 512          # nq tile (free dim of most matmuls)
NTILES = NQ // TQ  # 8 per batch

KQ = DQ // 128   # 4 contraction chunks for Wq
CI = INNER // 128  # 4 inner chunks


def _build_nc():
    nc = bacc.Bacc("TRN2", target_bir_lowering=False, debug=False)

    xT_l = nc.dram_tensor("xT_l", [BLOC, DQ, NQ], F16, kind="ExternalInput")
    kt_l = nc.dram_tensor("kt_l", [128, CI, BLOC * NK], F16,
                          kind="ExternalInput")
    v_l = nc.dram_tensor("v_l", [NK, BLOC, INNER], F16, kind="ExternalInput")
    # wq/wo uploaded pre-rearranged to [p, k, n] so the DMA is contiguous
    wq = nc.dram_tensor("wq", [128, KQ, INNER], F16, kind="ExternalInput")
    wo = nc.dram_tensor("wo", [128, CI, DQ], F16, kind="ExternalInput")
    ones77 = nc.dram_tensor("ones77", [NK, 64], F16, kind="ExternalInput")
    out_l = nc.dram_tensor("out_l", [BLOC, NQ, DQ], F16, kind="ExternalOutput")

    with tile.TileContext(nc) as tc:
        with (
            tc.tile_pool(name="consts", bufs=1) as consts,
            tc.tile_pool(name="xp", bufs=3) as xp,
            tc.tile_pool(name="ep", bufs=12) as ep,
            tc.tile_pool(name="rp", bufs=4) as rp,
            tc.tile_pool(name="op", bufs=2) as op,
            tc.tile_pool(name="fp", bufs=2) as fp,
            tc.tile_pool(name="ps", bufs=8, space="PSUM") as ps,
        ):
            # ---- DMAs: tile-0 Q-proj critical path first ----
            xT0_sb = xp.tile([128, KQ, TQ], F16, tag="xT")
            nc.sync.dma_start(
                xT0_sb[:],
                xT_l[0].rearrange("(c p) n -> p c n", p=128)[:, :, 0:TQ])
            wq_sb = consts.tile([128, KQ, INNER], F16)
            nc.scalar.dma_start(wq_sb[:], wq[:])
            kt_sb = consts.tile([128, CI, BLOC * NK], F16)
            nc.sync.dma_start(kt_sb[:], kt_l[:])
            v_sb = consts.tile([NK, BLOC, INNER], F16)
            nc.sync.dma_start(v_sb[:], v_l[:])
            ones77_sb = consts.tile([NK, 64], F16)
            nc.gpsimd.dma_start(ones77_sb[:], ones77[:])
            wo_sb = consts.tile([128, CI, DQ], F16)
            nc.scalar.dma_start(wo_sb[:], wo[:])

            # ---- main loop over (batch, nq tile) ----
            def process_tile(idx, b, nq0, tq, xT_sb):
                nj = tq // 128
                # Q^T = Wq^T @ x^T (Wq pre-scaled by 1/8 on host), with the
                # S^T matmuls for head pair g interleaved right after Q chunk
                # g+1 so the scalar exp chain starts ~2us earlier and the
                # den/O matmuls never wait on it. qt psum->sbuf copies run on
                # the vector engine (the scalar engine is exp-saturated in
                # this phase).
                qt_sb = xp.tile([128, CI, tq], F16, tag="qt",
                                name=f"qt_{idx}")
                e_sbs = []

                def emit_s_pair(g):
                    for h in (2 * g, 2 * g + 1):
                        c, r = h // 2, (h % 2) * 64
                        s_ps = ps.tile([NK, tq], F32, tag="ps",
                                       name=f"sps_{idx}_{h}")
                        nc.tensor.matmul(
                            s_ps[:],
                            kt_sb[r:r + DH, c, b * NK:(b + 1) * NK],
                            qt_sb[r:r + DH, c, :])
                        e_sb = ep.tile([NK, tq], F16, tag="expS",
                                       name=f"e_{idx}_{h}")
                        nc.scalar.activation(
                            e_sb[:], s_ps[:],
                            mybir.ActivationFunctionType.Exp)
                        e_sbs.append(e_sb)

                ot_sb = op.tile([128, CI, tq], F16, tag="ot",
                                name=f"ot_{idx}")

                def emit_pair(g):
                    # dens first so the vector engine's recip->mul chain for
                    # this pair starts two matmuls earlier.
                    o2_ps = ps.tile([128, tq], F32, tag="ps",
                                    name=f"ops_{idx}_{g}")
                    d_ps = ps.tile([128, tq], F32, tag="ps",
                                   name=f"dps_{idx}_{g}")
                    for half in range(2):
                        nc.tensor.matmul(
                            d_ps[half * 64:(half + 1) * 64, :],
                            ones77_sb[:], e_sbs[2 * g + half][:],
                            tile_position=(0, half * 64))
                    for half in range(2):
                        h = 2 * g + half
                        nc.tensor.matmul(
                            o2_ps[half * 64:(half + 1) * 64, :],
                            v_sb[:, b, h * DH:(h + 1) * DH],
                            e_sbs[h][:])
                    rec_sb = rp.tile([128, tq], F32, tag="rec",
                                     name=f"rec_{idx}_{g}")
                    nc.vector.reciprocal_approx_fast(rec_sb[:], d_ps[:])
                    nc.vector.tensor_mul(
                        ot_sb[:, g, :], o2_ps[:], rec_sb[:])

                for c in range(CI):
                    q_ps = ps.tile([128, tq], F32, tag="ps",
                                   name=f"qps_{idx}_{c}")
                    for k in range(KQ):
                        nc.tensor.matmul(
                            q_ps[:], wq_sb[:, k, c * 128:(c + 1) * 128],
                            xT_sb[:, k, :],
                            start=(k == 0), stop=(k == KQ - 1))
                    nc.vector.tensor_copy(qt_sb[:, c, :], q_ps[:])
                    if c >= 1:
                        emit_s_pair(c - 1)
                # heads 6,7 wait on the qt c3 copy; slot pair 0's den/O
                # matmuls in between so the PE never idles on that copy.
                emit_pair(0)
                emit_s_pair(3)
                for g in range(1, H // 2):
                    emit_pair(g)

                # out = (O^T).T @ Wo + bo; c-major accumulation so each
                # pair's contribution starts as soon as its mul lands.
                f_pss = [ps.tile([128, DQ], F32, tag="ps",
                                 name=f"fps_{idx}_{j}")
                         for j in range(nj)]
                for c in range(CI):
                    for j in range(nj):
                        nc.tensor.matmul(
                            f_pss[j][:],
                            ot_sb[:, c, j * 128:(j + 1) * 128],
                            wo_sb[:, c, :],
                            start=(c == 0), stop=(c == CI - 1))
                # f32->f16 cast on the scalar engine (idle in this phase);
                # the +bo bias is applied on host after gathering. On the
                # final tile, split casts across scalar and vector to halve
                # the pipeline drain.
                last = idx == len(tiles) - 1
                f_sb = fp.tile([128, nj, DQ], F16, tag="fin",
                               name=f"fin_{idx}")
                for j in range(nj):
                    if last and j % 2 == 1:
                        nc.vector.tensor_copy(f_sb[:, j, :], f_pss[j][:])
                    else:
                        nc.scalar.activation(
                            f_sb[:, j, :], f_pss[j][:],
                            mybir.ActivationFunctionType.Copy)
                    st_eng = [nc.sync, nc.scalar, nc.gpsimd][
                        (nj * idx + j) % 3]
                    st_eng.dma_start(
                        out_l[b, nq0 + j * 128:nq0 + (j + 1) * 128, :]
                        .rearrange("p n -> p n"),
                        f_sb[:, j, :])

            tiles = [(b, t * TQ, TQ)
                     for b in range(BLOC) for t in range(NTILES)]

            for idx, (b, nq0, tq) in enumerate(tiles):
                if idx == 0:
                    xT_sb = xT0_sb
                else:
                    xT_sb = xp.tile([128, KQ, tq], F16, tag="xT",
                                    name=f"xT_{idx}")
                    xT_src = xT_l[b].rearrange("(c p) n -> p c n", p=128)[
                        :, :, nq0:nq0 + tq]
                    if idx % 2 == 0:
                        nc.scalar.dma_start(xT_sb[:], xT_src)
                    else:
                        nc.sync.dma_start(xT_sb[:], xT_src)
                process_tile(idx, b, nq0, tq, xT_sb)

    nc.compile()
    return nc


_NC_CACHE = {}


def _get_nc():
    if "nc" not in _NC_CACHE:
        _NC_CACHE["nc"] = _build_nc()
    return _NC_CACHE["nc"]


def _make_in_maps(x, context, Wq, Wk, Wv, Wo, bo):
    f = np.float32
    ctx32 = np.ascontiguousarray(context, dtype=f)
    K_all = ctx32 @ np.asarray(Wk, dtype=f)   # [B, NK, INNER]
    V_all = ctx32 @ np.asarray(Wv, dtype=f)   # [B, NK, INNER]
    def fold_w(w, kdim, n):
        # [kdim*128, n] -> [128, kdim, n] contiguous
        return np.ascontiguousarray(
            w.reshape(kdim, 128, n).transpose(1, 0, 2))
    shared = {
        "wq": fold_w((np.asarray(Wq, dtype=f) * np.float32(SCALE)
                      ).astype(np.float16), KQ, INNER),
        "wo": fold_w(np.asarray(Wo, dtype=f).astype(np.float16), CI, DQ),
        "ones77": np.ones((NK, 64), dtype=np.float16),
    }
    in_maps = []
    for i in range(NCORES):
        m = dict(shared)
        m["xT_l"] = np.ascontiguousarray(
            np.asarray(x[BLOC * i:BLOC * (i + 1)], dtype=f).transpose(
                0, 2, 1)).astype(np.float16)
        K_c = K_all[BLOC * i:BLOC * (i + 1)]  # [BLOC, NK, INNER]
        # kt[p, c, b*NK+k] = K[b, k, c*128+p]
        m["kt_l"] = np.ascontiguousarray(
            K_c.transpose(2, 0, 1).reshape(CI, 128, BLOC, NK).transpose(
                1, 0, 2, 3).reshape(128, CI, BLOC * NK)).astype(np.float16)
        V_c = V_all[BLOC * i:BLOC * (i + 1)]
        m["v_l"] = np.ascontiguousarray(
            V_c.transpose(1, 0, 2)).astype(np.float16)
        in_maps.append(m)
    return in_maps


def run(x, context, Wq, Wk, Wv, Wo, bo, trace=False, **trace_kwargs):
    nc = _get_nc()
    in_maps = _make_in_maps(x, context, Wq, Wk, Wv, Wo, bo)
    res = run_bass_kernel_spmd(
        nc, in_maps, list(range(NCORES)), trace=trace, **trace_kwargs)
    out = np.concatenate(
        [np.asarray(res.results[i]["out_l"]) for i in range(NCORES)], axis=0)
    return out.astype(np.float32) + np.asarray(bo, dtype=np.float32), res


def kernel(x, context, Wq, Wk, Wv, Wo, bo):
    out, _ = run(x, context, Wq, Wk, Wv, Wo, bo, trace=False)
    return out


# revision 44
# speedup vs baseline: 1.0339x; 1.0019x over previous
"""CrossAttention kernel for 8 Trainium2 NeuronCores.

Data-parallel over batch: B=16 batches -> 2 per core. Each core computes the
full cross-attention for its 2 batches; outputs are concatenated on host.

K = ctx@Wk and V = ctx@Wv are tiny (0.15% of FLOPs) and computed on host
during input marshaling, uploaded pre-transposed in the layouts the PE wants.

Per-core dataflow (all matmuls f16 in, fp32 psum, full-rate on the PE):
  x      [2,4096,512] --host transpose--> x^T tiles [512, nq] (f16)
  Q^T    = Wq^T @ x^T            (Wq pre-scaled by 1/8 on host)
  S^T    = (K_h^T).T @ Q_h^T     [77, nq]  per head
  expS   = exp(S^T)              (no max-subtract; |S| <~ 6 so fp32 exp safe)
  den    = ones^T @ expS         [64, nq] per head, pairs packed [128, nq]
  O^T    = V_h.T @ expS          [64, nq], head pairs packed into [128, nq]
  ot     = O^T * (1/den)         (DVE recip + mul, per pair)
  out    = (O^T).T @ Wo + bo     [nq, 512], stored f16
"""

import os
import sys

for _p in ("/opt/trn_rl_repo",):
    if _p not in sys.path:
        sys.path.insert(0, _p)

import numpy as np

import concourse.bass as bass
import concourse.bass_isa as bass_isa
import concourse.bacc as bacc
import concourse.mybir as mybir
import concourse.tile as tile
from concourse.bass_utils import run_bass_kernel_spmd

# Problem constants (hardcoded per contract)
B, NQ, NK = 16, 4096, 77
DQ, DC = 512, 768
H, DH = 8, 64
INNER = H * DH  # 512
SCALE = DH ** -0.5  # 1/8
NCORES = 8
BLOC = B // NCORES  # 2 batches per core

F32 = mybir.dt.float32
F16 = mybir.dt.float16

TQ = 512          # nq tile (free dim of most matmuls)
NTILES = NQ // TQ  # 8 per batch

KQ = DQ // 128   # 4 contraction chunks for Wq
CI = INNER // 128  # 4 inner chunks


def _build_nc():
    nc = bacc.Bacc("TRN2", target_bir_lowering=False, debug=False)

    xT_l = nc.dram_tensor("xT_l", [BLOC, DQ, NQ], F16, kind="ExternalInput")
    kt_l = nc.dram_tensor("kt_l", [128, CI, BLOC * NK], F16,
                          kind="ExternalInput")
    v_l = nc.dram_tensor("v_l", [NK, BLOC, INNER], F16, kind="ExternalInput")
    # wq/wo uploaded pre-rearranged to [p, k, n] so the DMA is contiguous
    wq = nc.dram_tensor("wq", [128, KQ, INNER], F16, kind="ExternalInput")
    wo = nc.dram_tensor("wo", [128, CI, DQ], F16, kind="ExternalInput")
    ones77 = nc.dram_tensor("ones77", [NK, 64], F16, kind="ExternalInput")
    out_l = nc.dram_tensor("out_l", [BLOC, NQ, DQ], F16, kind="ExternalOutput")

    with tile.TileContext(nc) as tc:
        with (
            tc.tile_pool(name="consts", bufs=1) as consts,
            tc.tile_pool(name="xp", bufs=3) as xp,
            tc.tile_pool(name="ep", bufs=12) as ep,
            tc.tile_pool(name="rp", bufs=4) as rp,
            tc.tile_pool(name="op", bufs=2) as op,
            tc.tile_pool(name="fp", bufs=2) as fp,
            tc.tile_pool(name="ps", bufs=8, space="PSUM") as ps,
        ):
            # ---- DMAs: tile-0 Q-proj critical path first ----
            xT0_sb = xp.tile([128, KQ, TQ], F16, tag="xT")
            nc.sync.dma_start(
                xT0_sb[:],
                xT_l[0].rearrange("(c p) n -> p c n", p=128)[:, :, 0:TQ])
            wq_sb = consts.tile([128, KQ, INNER], F16)
            nc.scalar.dma_start(wq_sb[:], wq[:])
            kt_sb = consts.tile([128, CI, BLOC * NK], F16)
            nc.sync.dma_start(kt_sb[:], kt_l[:])
            v_sb = consts.tile([NK, BLOC, INNER], F16)
            nc.sync.dma_start(v_sb[:], v_l[:])
            ones77_sb = consts.tile([NK, 64], F16)
            nc.gpsimd.dma_start(ones77_sb[:], ones77[:])
            wo_sb = consts.tile([128, CI, DQ], F16)
            nc.scalar.dma_start(wo_sb[:], wo[:])

            # ---- main loop over (batch, nq tile) ----
            def process_tile(idx, b, nq0, tq, xT_sb):
                nj = tq // 128
                # Q^T = Wq^T @ x^T (Wq pre-scaled by 1/8 on host), with the
                # S^T matmuls for head pair g interleaved right after Q chunk
                # g+1 so the scalar exp chain starts ~2us earlier and the
                # den/O matmuls never wait on it. qt psum->sbuf copies run on
                # the vector engine (the scalar engine is exp-saturated in
                # this phase).
                qt_sb = xp.tile([128, CI, tq], F16, tag="qt",
                                name=f"qt_{idx}")
                e_sbs = []

                def emit_s_pair(g):
                    for h in (2 * g, 2 * g + 1):
                        c, r = h // 2, (h % 2) * 64
                        s_ps = ps.tile([NK, tq], F32, tag="ps",
                                       name=f"sps_{idx}_{h}")
                        nc.tensor.matmul(
                            s_ps[:],
                            kt_sb[r:r + DH, c, b * NK:(b + 1) * NK],
                            qt_sb[r:r + DH, c, :])
                        e_sb = ep.tile([NK, tq], F16, tag="expS",
                                       name=f"e_{idx}_{h}")
                        nc.scalar.activation(
                            e_sb[:], s_ps[:],
                            mybir.ActivationFunctionType.Exp)
                        e_sbs.append(e_sb)

                ot_sb = op.tile([128, CI, tq], F16, tag="ot",
                                name=f"ot_{idx}")

                def emit_pair(g):
                    # dens first so the vector engine's recip->mul chain for
                    # this pair starts two matmuls earlier.
                    o2_ps = ps.tile([128, tq], F32, tag="ps",
                                    name=f"ops_{idx}_{g}")
                    d_ps = ps.tile([128, tq], F32, tag="ps",
                                   name=f"dps_{idx}_{g}")
                    for half in range(2):
                        nc.tensor.matmul(
                            d_ps[half * 64:(half + 1) * 64, :],
                            ones77_sb[:], e_sbs[2 * g + half][:],
                            tile_position=(0, half * 64))
                    for half in range(2):
                        h = 2 * g + half
                        nc.tensor.matmul(
                            o2_ps[half * 64:(half + 1) * 64, :],
                            v_sb[:, b, h * DH:(h + 1) * DH],
                            e_sbs[h][:])
                    rec_sb = rp.tile([128, tq], F32, tag="rec",
                                     name=f"rec_{idx}_{g}")
                    nc.vector.reciprocal_approx_fast(rec_sb[:], d_ps[:])
                    nc.vector.tensor_mul(
                        ot_sb[:, g, :], o2_ps[:], rec_sb[:])

                for c in range(CI):
                    q_ps = ps.tile([128, tq], F32, tag="ps",
                                   name=f"qps_{idx}_{c}")
                    for k in range(KQ):
                        nc.tensor.matmul(
                            q_ps[:], wq_sb[:, k, c * 128:(c + 1) * 128],
                            xT_sb[:, k, :],
                            start=(k == 0), stop=(k == KQ - 1))
                    nc.vector.tensor_copy(qt_sb[:, c, :], q_ps[:])
                    if c >= 1:
                        emit_s_pair(c - 1)
                # heads 6,7 wait on the qt c3 copy; slot pair 0's den/O
                # matmuls in between so the PE never idles on that copy.
                emit_pair(0)
                emit_s_pair(3)
                for g in range(1, H // 2):
                    emit_pair(g)

                # out = (O^T).T @ Wo + bo; c-major accumulation so each
                # pair's contribution starts as soon as its mul lands.
                f_pss = [ps.tile([128, DQ], F32, tag="ps",
                                 name=f"fps_{idx}_{j}")
                         for j in range(nj)]
                for c in range(CI):
                    for j in range(nj):
                        nc.tensor.matmul(
                            f_pss[j][:],
                            ot_sb[:, c, j * 128:(j + 1) * 128],
                            wo_sb[:, c, :],
                            start=(c == 0), stop=(c == CI - 1))
                # f32->f16 cast on the scalar engine (idle in this phase);
                # the +bo bias is applied on host after gathering. On the
                # final tile, split casts across scalar and vector to halve
                # the pipeline drain.
                last = idx == len(tiles) - 1
                f_sb = fp.tile([128, nj, DQ], F16, tag="fin",
                               name=f"fin_{idx}")
                for j in range(nj):
                    if last and j % 2 == 1:
                        nc.vector.tensor_copy(f_sb[:, j, :], f_pss[j][:])
                    else:
                        nc.scalar.activation(
                            f_sb[:, j, :], f_pss[j][:],
                            mybir.ActivationFunctionType.Copy)
                    st_eng = [nc.sync, nc.scalar][(nj * idx + j) % 2]
                    st_eng.dma_start(
                        out_l[b, nq0 + j * 128:nq0 + (j + 1) * 128, :]
                        .rearrange("p n -> p n"),
                        f_sb[:, j, :])

            tiles = [(b, t * TQ, TQ)
                     for b in range(BLOC) for t in range(NTILES)]

            for idx, (b, nq0, tq) in enumerate(tiles):
                if idx == 0:
                    xT_sb = xT0_sb
                else:
                    xT_sb = xp.tile([128, KQ, tq], F16, tag="xT",
                                    name=f"xT_{idx}")
                    xT_src = xT_l[b].rearrange("(c p) n -> p c n", p=128)[
                        :, :, nq0:nq0 + tq]
                    if idx % 2 == 0:
                        nc.scalar.dma_start(xT_sb[:], xT_src)
                    else:
                        nc.sync.dma_start(xT_sb[:], xT_src)
                process_tile(idx, b, nq0, tq, xT_sb)

    nc.compile()
    return nc


_NC_CACHE = {}


def _get_nc():
    if "nc" not in _NC_CACHE:
        _NC_CACHE["nc"] = _build_nc()
    return _NC_CACHE["nc"]


def _make_in_maps(x, context, Wq, Wk, Wv, Wo, bo):
    f = np.float32
    ctx32 = np.ascontiguousarray(context, dtype=f)
    K_all = ctx32 @ np.asarray(Wk, dtype=f)   # [B, NK, INNER]
    V_all = ctx32 @ np.asarray(Wv, dtype=f)   # [B, NK, INNER]
    def fold_w(w, kdim, n):
        # [kdim*128, n] -> [128, kdim, n] contiguous
        return np.ascontiguousarray(
            w.reshape(kdim, 128, n).transpose(1, 0, 2))
    shared = {
        "wq": fold_w((np.asarray(Wq, dtype=f) * np.float32(SCALE)
                      ).astype(np.float16), KQ, INNER),
        "wo": fold_w(np.asarray(Wo, dtype=f).astype(np.float16), CI, DQ),
        "ones77": np.ones((NK, 64), dtype=np.float16),
    }
    in_maps = []
    for i in range(NCORES):
        m = dict(shared)
        m["xT_l"] = np.ascontiguousarray(
            np.asarray(x[BLOC * i:BLOC * (i + 1)], dtype=f).transpose(
                0, 2, 1)).astype(np.float16)
        K_c = K_all[BLOC * i:BLOC * (i + 1)]  # [BLOC, NK, INNER]
        # kt[p, c, b*NK+k] = K[b, k, c*128+p]
        m["kt_l"] = np.ascontiguousarray(
            K_c.transpose(2, 0, 1).reshape(CI, 128, BLOC, NK).transpose(
                1, 0, 2, 3).reshape(128, CI, BLOC * NK)).astype(np.float16)
        V_c = V_all[BLOC * i:BLOC * (i + 1)]
        m["v_l"] = np.ascontiguousarray(
            V_c.transpose(1, 0, 2)).astype(np.float16)
        in_maps.append(m)
    return in_maps


def run(x, context, Wq, Wk, Wv, Wo, bo, trace=False, **trace_kwargs):
    nc = _get_nc()
    in_maps = _make_in_maps(x, context, Wq, Wk, Wv, Wo, bo)
    res = run_bass_kernel_spmd(
        nc, in_maps, list(range(NCORES)), trace=trace, **trace_kwargs)
    out = np.concatenate(
        [np.asarray(res.results[i]["out_l"]) for i in range(NCORES)], axis=0)
    return out.astype(np.float32) + np.asarray(bo, dtype=np.float32), res


def kernel(x, context, Wq, Wk, Wv, Wo, bo):
    out, _ = run(x, context, Wq, Wk, Wv, Wo, bo, trace=False)
    return out
